# revision 31
# baseline (speedup 1.0000x reference)
"""ALNN layer on 8 TRN2 NeuronCores (Bass/Tile, SPMD — no collectives).

Math (per reference):
  ref_r = linspace(0, 48, 64);  a_r = relu(alpha_r)
  e[b,r,l,d]  = exp(-a_r * |T[b,l,d] - ref_r|)
  intensity   = relu(X * e) = relu(X) * e            (e > 0 always)
  p[b,r,l,d]  = w0*X + w1*relu(X)*e + w2*M + w3*DT + w4*P + 5*b_t[r,l,d]
  h           = relu(p)
  out[b,r,d]  = relu( sum_l w_v[r,l,d]*h + 128*b_v[r,d] )

Design (measured on HW, ~112us):
- Shard R=64 across the 8 cores (8 r each); inputs replicated; host
  concatenates the per-core [B, 8, D] outputs. No cross-core traffic.
- Layout: partition = L (=128), free = (r-pair, b, d); the main loop
  processes r in PAIRS so every DVE op runs at FD 3072 (bf16 2x_1P mode,
  ~1.75us/op — per-instruction overhead amortized).
- All 12 tensor-ops per pair run on VectorE in bf16. GPSIMD is deliberately
  unused: it shares SBUF ports with DVE and starves (25us+ op outliers).
- Weights are host-expanded over 16 b and consumed via 5-D APs with a
  0-stride repeat dim ([l, r2, rep2, 16, d]) — broadcast-operand TTs on DVE
  otherwise fall off the fast mode (4x slower). Per-channel DMA tiles let
  each weight slice gate only its own mul.
- ACT (ScalarE) computes dist=Abs(T-ref_r) and e=Exp(-a_r*dist) with per-r
  [P,1] bias/scale operands, plus the relu. f32 dist path for accuracy.
- The sum over L runs on the TensorEngine: per r, one-hot-column lhsT
  matmuls accumulate wv*h into a shared PSUM tile [8, b-chunk, D]; the
  128*b_v bias is folded in by an identity-rhs matmul that opens each
  accumulation group. Epilogue: relu(psum) on ACT, chunked DMA out.
"""
import sys

import numpy as np

if "/opt/trn_rl_repo" not in sys.path:
    sys.path.insert(0, "/opt/trn_rl_repo")

import ml_dtypes

from concourse import bacc, mybir
import concourse.tile as tile
from concourse.bass_utils import run_bass_kernel_spmd

BF16 = ml_dtypes.bfloat16
B, L, D = 32, 128, 48
R = 64
RL = R // 8  # r per core
INIT_TIME, MAX_TS = 0.0, 48.0

_CACHE = {}


def _build():
    nc = bacc.Bacc("TRN2", target_bir_lowering=False, debug=False, num_devices=8)
    f32, bf16 = mybir.dt.float32, mybir.dt.bfloat16
    AF = mybir.ActivationFunctionType

    # DRAM parameters (per-core shards / replicas)
    dTt = nc.dram_tensor("Tt", [L, B, D], f32, kind="ExternalInput").ap()
    dX = nc.dram_tensor("Xb", [L, B, D], bf16, kind="ExternalInput").ap()
    dXP = nc.dram_tensor("XPb", [L, B, D], bf16, kind="ExternalInput").ap()
    dM = nc.dram_tensor("Mb", [L, B, D], bf16, kind="ExternalInput").ap()
    dDT = nc.dram_tensor("DTb", [L, B, D], bf16, kind="ExternalInput").ap()
    dP = nc.dram_tensor("Pb", [L, B, D], bf16, kind="ExternalInput").ap()
    dWX = nc.dram_tensor("WX", [RL // 2, L, 7, 2, 1, D], bf16, kind="ExternalInput").ap()
    dRN = nc.dram_tensor("RN", [L, RL], f32, kind="ExternalInput").ap()
    dAN = nc.dram_tensor("AN", [L, RL], f32, kind="ExternalInput").ap()
    dBV = nc.dram_tensor("BVl", [D, RL], bf16, kind="ExternalInput").ap()
    dID = nc.dram_tensor("ID48", [D, D], bf16, kind="ExternalInput").ap()
    dOUT = nc.dram_tensor("out", [B, RL, D], f32, kind="ExternalOutput").ap()

    NCH = 4  # psum chunks of 8 b each (8*48 = 384 floats < 512/bank)
    BC = B // NCH

    with tile.TileContext(nc) as tc:
        with (
            tc.tile_pool(name="const", bufs=1) as cpool,
            tc.tile_pool(name="work", bufs=2) as wpool,
            tc.tile_pool(name="psum", bufs=1, space="PSUM") as ppool,
            tc.tile_pool(name="outp", bufs=1) as opool,
        ):
            # ---- load persistent inputs (X first: XP gates the first DVE op)
            tX = cpool.tile([L, B, D], bf16, tag="X")
            nc.sync.dma_start(tX[:], dX)
            tXP = cpool.tile([L, B, D], bf16, tag="XP")
            nc.sync.dma_start(tXP[:], dXP)
            # pair-0 weights early: the first DVE ops gate on them
            wt0 = wpool.tile([L, 7, 2, 1, D], bf16, tag="wt", name="wt0", bufs=2)
            for k in (1, 0, 4):
                nc.sync.dma_start(wt0[:, k], dWX[0, :, k])
            tT = cpool.tile([L, B, D], f32, tag="T")
            nc.sync.dma_start(tT[:], dTt)
            tM = cpool.tile([L, B, D], bf16, tag="M")
            nc.sync.dma_start(tM[:], dM)
            tDT = cpool.tile([L, B, D], bf16, tag="DT")
            nc.sync.dma_start(tDT[:], dDT)
            tP = cpool.tile([L, B, D], bf16, tag="P")
            nc.sync.dma_start(tP[:], dP)
            tRN = cpool.tile([L, RL], f32, tag="RN")
            nc.sync.dma_start(tRN[:], dRN)
            tAN = cpool.tile([L, RL], f32, tag="AN")
            nc.sync.dma_start(tAN[:], dAN)
            tBV = cpool.tile([D, RL], bf16, tag="BV")
            nc.sync.dma_start(tBV[:], dBV)
            tID = cpool.tile([D, D], bf16, tag="ID")
            nc.sync.dma_start(tID[:], dID)


            # one-hot lhsT matrices: oh[l, j, m] = (m == j)
            tOH = cpool.tile([L, RL, RL], bf16, tag="OH")
            nc.vector.memset(tOH[:], 0.0)
            for j in range(RL):
                nc.vector.memset(tOH[:, j, j : j + 1], 1.0)

            # psum accumulators, one bank-sized chunk of (b, d) each
            psc = [
                ppool.tile([RL, BC, D], mybir.dt.float32, tag=f"ps{c}", name=f"ps{c}")
                for c in range(NCH)
            ]
            # open each accumulation group with the bias term:
            # ps[r, b, d] = 128*b_v[r, d] via lhsT=BVl[d', r], rhs=Id[d', (b d)]
            for c in range(NCH):
                nc.tensor.matmul(
                    psc[c][:],
                    tBV[:],
                    tID[:, None, :].to_broadcast((D, BC, D)),
                    start=True,
                    stop=False,
                )

            H = 1    # weights not expanded; pure 0-stride broadcast over b; 0-stride rep covers all 32
            NP = RL // 2  # r-pairs per core
            S5 = (L, 2, 32, H, D)
            g2 = lambda ap: ap.rearrange("l r (g h) d -> l r g h d", g=32)
            for jj in range(NP):
                if jj == 0:
                    wt = wt0
                    for k in (2, 3, 5, 6):
                        nc.sync.dma_start(wt[:, k], dWX[jj, :, k])
                else:
                    wt = wpool.tile([L, 7, 2, H, D], bf16, tag="wt", name=f"wt{jj}", bufs=2)
                    for k in (1, 0, 4, 2, 3, 5, 6):
                        nc.sync.dma_start(wt[:, k], dWX[jj, :, k])
                wop = lambda k: wt[:, k, :, None, :, :].to_broadcast(S5)

                dist = wpool.tile([L, 2, B, D], f32, tag="dist", bufs=2)
                ebf = wpool.tile([L, 2, B, D], bf16, tag="ebf", bufs=2)
                for rr in range(2):
                    j = 2 * jj + rr
                    nc.scalar.activation(
                        dist[:, rr], tT[:], AF.Abs, bias=tRN[:, j : j + 1]
                    )
                    nc.scalar.activation(
                        ebf[:, rr], dist[:, rr], AF.Exp, scale=tAN[:, j : j + 1]
                    )

                rep = lambda tin: g2(tin[:, None].to_broadcast((L, 2, B, D)))
                q = wpool.tile([L, 2, B, D], bf16, tag="q", bufs=2)
                nc.vector.tensor_mul(g2(q[:]), rep(tXP), wop(1))
                a0 = wpool.tile([L, 2, B, D], bf16, tag="aa", name=f"a0_{jj}", bufs=4)
                nc.vector.tensor_mul(g2(a0[:]), rep(tX), wop(0))
                a2 = wpool.tile([L, 2, B, D], bf16, tag="aa", name=f"a2_{jj}", bufs=4)
                nc.vector.tensor_mul(g2(a2[:]), rep(tM), wop(2))
                a3 = wpool.tile([L, 2, B, D], bf16, tag="aa", name=f"a3_{jj}", bufs=4)
                nc.vector.tensor_mul(g2(a3[:]), rep(tDT), wop(3))
                a4 = wpool.tile([L, 2, B, D], bf16, tag="aa", name=f"a4_{jj}", bufs=4)
                nc.vector.tensor_mul(g2(a4[:]), rep(tP), wop(4))

                t = wpool.tile([L, 2, B, D], bf16, tag="t", bufs=2)
                nc.vector.tensor_mul(t[:], q[:], ebf[:])
                s1 = wpool.tile([L, 2, B, D], bf16, tag="s", name=f"s1_{jj}", bufs=3)
                nc.vector.tensor_add(s1[:], a0[:], a4[:])
                s2 = wpool.tile([L, 2, B, D], bf16, tag="s", name=f"s2_{jj}", bufs=3)
                nc.vector.tensor_add(s2[:], a2[:], a3[:])
                s3 = wpool.tile([L, 2, B, D], bf16, tag="s", name=f"s3_{jj}", bufs=3)
                nc.vector.tensor_add(s3[:], s1[:], s2[:])
                s4 = wpool.tile([L, 2, B, D], bf16, tag="s", name=f"s4_{jj}", bufs=3)
                nc.vector.tensor_add(g2(s4[:]), g2(s3[:]), wop(5))
                p = wpool.tile([L, 2, B, D], bf16, tag="s", name=f"p_{jj}", bufs=3)
                nc.vector.tensor_add(p[:], s4[:], t[:])

                h = wpool.tile([L, 2, B, D], bf16, tag="h", bufs=2)
                nc.scalar.activation(h[:], p[:], AF.Relu)
                wh = wpool.tile([L, 2, B, D], bf16, tag="wh", bufs=2)
                nc.vector.tensor_mul(g2(wh[:]), g2(h[:]), wop(6))

                for rr in range(2):
                    j = 2 * jj + rr
                    for c in range(NCH):
                        nc.tensor.matmul(
                            psc[c][:],
                            tOH[:, j, :],
                            wh[:, rr, c * BC : (c + 1) * BC, :],
                            start=False,
                            stop=(j == RL - 1),
                        )

            # epilogue: relu(psum) -> sbuf f32, DMA out per chunk
            outf = opool.tile([RL, B, D], mybir.dt.float32, tag="outf")
            dOUTt = dOUT.transpose([1, 0, 2])
            for c in range(NCH):
                cs = slice(c * BC, (c + 1) * BC)
                nc.scalar.activation(outf[:, cs, :], psc[c][:], AF.Relu)
                nc.sync.dma_start(dOUTt[:, cs, :], outf[:, cs, :])

    nc.compile()
    return nc


def _prep(X, T, M, DT, P, alpha, w_t, b_t, w_v, b_v):
    """Host-side shard prep: returns in_maps for the 8 cores."""
    X, T, M, DT, P, alpha, w_t, b_t, w_v, b_v = (
        np.asarray(a) for a in (X, T, M, DT, P, alpha, w_t, b_t, w_v, b_v)
    )
    refs = np.linspace(INIT_TIME, MAX_TS, R, dtype=np.float32)
    arelu = np.maximum(alpha.reshape(R).astype(np.float32), 0.0)

    Tt = np.ascontiguousarray(T.transpose(1, 0, 2)).astype(np.float32)
    Xb = np.ascontiguousarray(X.transpose(1, 0, 2)).astype(BF16)
    XPb = np.maximum(Xb, 0)
    Mb = np.ascontiguousarray(M.transpose(1, 0, 2)).astype(BF16)
    DTb = np.ascontiguousarray(DT.transpose(1, 0, 2)).astype(BF16)
    Pb = np.ascontiguousarray(P.transpose(1, 0, 2)).astype(BF16)
    id48 = np.eye(D, dtype=np.float32).astype(BF16)

    # WX[pair, l, k, rr, b16, d]: per r-pair, 7 weight channels expanded over
    # 16 b (b-independent; a 0-stride AP repeats them across all 32 b).
    wk_full = np.concatenate([w_t, 5.0 * b_t, w_v[..., None]], axis=3)  # [R, L, D, 7]
    in_maps = []
    for i in range(8):
        r0 = i * RL
        wx = wk_full[r0 : r0 + RL].transpose(1, 3, 0, 2)  # [L, 7, RL, D]
        wx = wx.reshape(L, 7, RL // 2, 2, D).transpose(2, 0, 1, 3, 4)  # [NP, L, 7, 2, D]
        wx = np.ascontiguousarray(
            np.broadcast_to(wx[:, :, :, :, None, :], (RL // 2, L, 7, 2, 1, D))
        ).astype(BF16)
        rn = np.broadcast_to(-refs[r0 : r0 + RL], (L, RL)).astype(np.float32)
        an = np.broadcast_to(-arelu[r0 : r0 + RL], (L, RL)).astype(np.float32)
        bvl = np.ascontiguousarray(
            (128.0 * b_v[r0 : r0 + RL, 0, :]).T
        ).astype(BF16)  # [D, RL]
        in_maps.append(
            {
                "Tt": Tt,
                "Xb": Xb,
                "XPb": XPb,
                "Mb": Mb,
                "DTb": DTb,
                "Pb": Pb,
                "WX": wx,
                "RN": np.ascontiguousarray(rn),
                "AN": np.ascontiguousarray(an),
                "BVl": bvl,
                "ID48": id48,
            }
        )
    return in_maps


def run(trace=False, **inputs):
    if "nc" not in _CACHE:
        _CACHE["nc"] = _build()
    nc = _CACHE["nc"]
    in_maps = _prep(**inputs)
    res = run_bass_kernel_spmd(nc, in_maps, core_ids=list(range(8)), trace=trace)
    out = np.empty((B, R, D), dtype=np.float32)
    for i in range(8):
        out[:, i * RL : (i + 1) * RL, :] = res.results[i]["out"]
    return out, res


def kernel(**inputs) -> np.ndarray:
    out, _ = run(trace=False, **inputs)
    return out


# revision 32
# speedup vs baseline: 1.1718x; 1.1718x over previous
"""ALNN layer on 8 TRN2 NeuronCores (Bass/Tile, SPMD — no collectives).

Math (per reference):
  ref_r = linspace(0, 48, 64);  a_r = relu(alpha_r)
  e[b,r,l,d]  = exp(-a_r * |T[b,l,d] - ref_r|)
  intensity   = relu(X * e) = relu(X) * e            (e > 0 always)
  p[b,r,l,d]  = w0*X + w1*relu(X)*e + w2*M + w3*DT + w4*P + 5*b_t[r,l,d]
  h           = relu(p)
  out[b,r,d]  = relu( sum_l w_v[r,l,d]*h + 128*b_v[r,d] )

Design (measured on HW, ~109us):
- Shard R=64 across the 8 cores (8 r each); inputs replicated; host
  concatenates the per-core [B, 8, D] outputs. No cross-core traffic.
- Layout: partition = L (=128), free = (r-pair, b, d); the main loop
  processes r in PAIRS so every DVE op runs at FD 3072 (bf16 2x_1P mode,
  ~1.75us/op — per-instruction overhead amortized).
- All 12 tensor-ops per pair run on VectorE in bf16. GPSIMD is deliberately
  unused: it shares SBUF ports with DVE and starves (25us+ op outliers).
- Weights are consumed via 5-D APs [l, r2, (b:32 stride-0), 1, d]: with the
  r-pair dim leading, the pure-broadcast operand keeps DVE's 2x mode (plain
  per-r [l, (b:0), d] broadcasts measured ~4x slower). Weight DMA is the raw
  0.7MB per core, split per (pair, channel) so each slice gates only its mul.
- ACT (ScalarE) computes dist=Abs(T-ref_r) and e=Exp(-a_r*dist) with per-r
  [P,1] bias/scale operands, plus the relu. f32 dist path for accuracy.
- The sum over L runs on the TensorEngine: per r, one-hot-column lhsT
  matmuls accumulate wv*h into a shared PSUM tile [8, b-chunk, D]; the
  128*b_v bias is folded in by an identity-rhs matmul that opens each
  accumulation group. Epilogue: relu(psum) on ACT, chunked DMA out.
"""
import sys

import numpy as np

if "/opt/trn_rl_repo" not in sys.path:
    sys.path.insert(0, "/opt/trn_rl_repo")

import ml_dtypes

from concourse import bacc, mybir
import concourse.tile as tile
from concourse.bass_utils import run_bass_kernel_spmd

BF16 = ml_dtypes.bfloat16
B, L, D = 32, 128, 48
R = 64
RL = R // 8  # r per core
INIT_TIME, MAX_TS = 0.0, 48.0

_CACHE = {}


def _build():
    nc = bacc.Bacc("TRN2", target_bir_lowering=False, debug=False, num_devices=8)
    f32, bf16 = mybir.dt.float32, mybir.dt.bfloat16
    AF = mybir.ActivationFunctionType

    # DRAM parameters (per-core shards / replicas)
    dTt = nc.dram_tensor("Tt", [L, B, D], f32, kind="ExternalInput").ap()
    dX = nc.dram_tensor("Xb", [L, B, D], bf16, kind="ExternalInput").ap()
    dXP = nc.dram_tensor("XPb", [L, B, D], bf16, kind="ExternalInput").ap()
    dM = nc.dram_tensor("Mb", [L, B, D], bf16, kind="ExternalInput").ap()
    dDT = nc.dram_tensor("DTb", [L, B, D], bf16, kind="ExternalInput").ap()
    dP = nc.dram_tensor("Pb", [L, B, D], bf16, kind="ExternalInput").ap()
    dWX = nc.dram_tensor("WX", [RL // 2, L, 7, 2, 1, D], bf16, kind="ExternalInput").ap()
    dRN = nc.dram_tensor("RN", [L, RL], f32, kind="ExternalInput").ap()
    dAN = nc.dram_tensor("AN", [L, RL], f32, kind="ExternalInput").ap()
    dBV = nc.dram_tensor("BVl", [D, RL], bf16, kind="ExternalInput").ap()
    dID = nc.dram_tensor("ID48", [D, D], bf16, kind="ExternalInput").ap()
    dOUT = nc.dram_tensor("out", [B, RL, D], f32, kind="ExternalOutput").ap()

    NCH = 4  # psum chunks of 8 b each (8*48 = 384 floats < 512/bank)
    BC = B // NCH

    with tile.TileContext(nc) as tc:
        with (
            tc.tile_pool(name="const", bufs=1) as cpool,
            tc.tile_pool(name="work", bufs=2) as wpool,
            tc.tile_pool(name="psum", bufs=1, space="PSUM") as ppool,
            tc.tile_pool(name="outp", bufs=1) as opool,
        ):
            # ---- load persistent inputs (X first: XP gates the first DVE op)
            tX = cpool.tile([L, B, D], bf16, tag="X")
            nc.sync.dma_start(tX[:], dX)
            tXP = cpool.tile([L, B, D], bf16, tag="XP")
            nc.sync.dma_start(tXP[:], dXP)
            # pair-0 weights early: the first DVE ops gate on them
            wt0 = wpool.tile([L, 7, 2, 1, D], bf16, tag="wt", name="wt0", bufs=2)
            for k in (1, 0, 4):
                nc.sync.dma_start(wt0[:, k], dWX[0, :, k])
            tT = cpool.tile([L, B, D], f32, tag="T")
            nc.sync.dma_start(tT[:], dTt)
            tM = cpool.tile([L, B, D], bf16, tag="M")
            nc.sync.dma_start(tM[:], dM)
            tDT = cpool.tile([L, B, D], bf16, tag="DT")
            nc.sync.dma_start(tDT[:], dDT)
            tP = cpool.tile([L, B, D], bf16, tag="P")
            nc.sync.dma_start(tP[:], dP)
            tRN = cpool.tile([L, RL], f32, tag="RN")
            nc.sync.dma_start(tRN[:], dRN)
            tAN = cpool.tile([L, RL], f32, tag="AN")
            nc.sync.dma_start(tAN[:], dAN)
            tBV = cpool.tile([D, RL], bf16, tag="BV")
            nc.sync.dma_start(tBV[:], dBV)
            tID = cpool.tile([D, D], bf16, tag="ID")
            nc.sync.dma_start(tID[:], dID)


            # one-hot lhsT matrices: oh[l, j, m] = (m == j)
            tOH = cpool.tile([L, RL, RL], bf16, tag="OH")
            nc.vector.memset(tOH[:], 0.0)
            for j in range(RL):
                nc.vector.memset(tOH[:, j, j : j + 1], 1.0)

            # psum accumulators, one bank-sized chunk of (b, d) each
            psc = [
                ppool.tile([RL, BC, D], mybir.dt.float32, tag=f"ps{c}", name=f"ps{c}")
                for c in range(NCH)
            ]
            # open each accumulation group with the bias term:
            # ps[r, b, d] = 128*b_v[r, d] via lhsT=BVl[d', r], rhs=Id[d', (b d)]
            for c in range(NCH):
                nc.tensor.matmul(
                    psc[c][:],
                    tBV[:],
                    tID[:, None, :].to_broadcast((D, BC, D)),
                    start=True,
                    stop=False,
                )

            H = 1    # weights not expanded; pure 0-stride broadcast over b; 0-stride rep covers all 32
            NP = RL // 2  # r-pairs per core
            S5 = (L, 2, 32, H, D)
            g2 = lambda ap: ap.rearrange("l r (g h) d -> l r g h d", g=32)
            for jj in range(NP):
                if jj == 0:
                    wt = wt0
                    for k in (2, 3, 5, 6):
                        nc.sync.dma_start(wt[:, k], dWX[jj, :, k])
                else:
                    wt = wpool.tile([L, 7, 2, H, D], bf16, tag="wt", name=f"wt{jj}", bufs=2)
                    for k in (1, 0, 4, 2, 3, 5, 6):
                        nc.sync.dma_start(wt[:, k], dWX[jj, :, k])
                wop = lambda k: wt[:, k, :, None, :, :].to_broadcast(S5)

                dist = wpool.tile([L, 2, B, D], f32, tag="dist", bufs=2)
                ebf = wpool.tile([L, 2, B, D], bf16, tag="ebf", bufs=2)
                for rr in range(2):
                    j = 2 * jj + rr
                    nc.scalar.activation(
                        dist[:, rr], tT[:], AF.Abs, bias=tRN[:, j : j + 1]
                    )
                    nc.scalar.activation(
                        ebf[:, rr], dist[:, rr], AF.Exp, scale=tAN[:, j : j + 1]
                    )

                rep = lambda tin: g2(tin[:, None].to_broadcast((L, 2, B, D)))
                q = wpool.tile([L, 2, B, D], bf16, tag="q", bufs=2)
                nc.vector.tensor_mul(g2(q[:]), rep(tXP), wop(1))
                a0 = wpool.tile([L, 2, B, D], bf16, tag="aa", name=f"a0_{jj}", bufs=4)
                nc.vector.tensor_mul(g2(a0[:]), rep(tX), wop(0))
                a2 = wpool.tile([L, 2, B, D], bf16, tag="aa", name=f"a2_{jj}", bufs=4)
                nc.vector.tensor_mul(g2(a2[:]), rep(tM), wop(2))
                a3 = wpool.tile([L, 2, B, D], bf16, tag="aa", name=f"a3_{jj}", bufs=4)
                nc.vector.tensor_mul(g2(a3[:]), rep(tDT), wop(3))
                a4 = wpool.tile([L, 2, B, D], bf16, tag="aa", name=f"a4_{jj}", bufs=4)
                nc.vector.tensor_mul(g2(a4[:]), rep(tP), wop(4))

                t = wpool.tile([L, 2, B, D], bf16, tag="t", bufs=2)
                nc.vector.tensor_mul(t[:], q[:], ebf[:])
                s1 = wpool.tile([L, 2, B, D], bf16, tag="s", name=f"s1_{jj}", bufs=3)
                nc.vector.tensor_add(s1[:], a0[:], a4[:])
                s2 = wpool.tile([L, 2, B, D], bf16, tag="s", name=f"s2_{jj}", bufs=3)
                nc.vector.tensor_add(s2[:], a2[:], a3[:])
                s3 = wpool.tile([L, 2, B, D], bf16, tag="s", name=f"s3_{jj}", bufs=3)
                nc.vector.tensor_add(s3[:], s1[:], s2[:])
                s4 = wpool.tile([L, 2, B, D], bf16, tag="s", name=f"s4_{jj}", bufs=3)
                nc.vector.tensor_add(g2(s4[:]), g2(s3[:]), wop(5))
                p = wpool.tile([L, 2, B, D], bf16, tag="s", name=f"p_{jj}", bufs=3)
                nc.vector.tensor_add(p[:], s4[:], t[:])

                h = wpool.tile([L, 2, B, D], bf16, tag="h", bufs=2)
                nc.scalar.activation(h[:], p[:], AF.Relu)
                wh = wpool.tile([L, 2, B, D], bf16, tag="wh", bufs=2)
                nc.vector.tensor_mul(g2(wh[:]), g2(h[:]), wop(6))

                for rr in range(2):
                    j = 2 * jj + rr
                    for c in range(NCH):
                        nc.tensor.matmul(
                            psc[c][:],
                            tOH[:, j, :],
                            wh[:, rr, c * BC : (c + 1) * BC, :],
                            start=False,
                            stop=(j == RL - 1),
                        )

            # epilogue: relu(psum) -> sbuf f32, DMA out per chunk
            outf = opool.tile([RL, B, D], mybir.dt.float32, tag="outf")
            dOUTt = dOUT.transpose([1, 0, 2])
            for c in range(NCH):
                cs = slice(c * BC, (c + 1) * BC)
                nc.scalar.activation(outf[:, cs, :], psc[c][:], AF.Relu)
                nc.sync.dma_start(dOUTt[:, cs, :], outf[:, cs, :])

    nc.compile()
    return nc


def _prep(X, T, M, DT, P, alpha, w_t, b_t, w_v, b_v):
    """Host-side shard prep: returns in_maps for the 8 cores."""
    X, T, M, DT, P, alpha, w_t, b_t, w_v, b_v = (
        np.asarray(a) for a in (X, T, M, DT, P, alpha, w_t, b_t, w_v, b_v)
    )
    refs = np.linspace(INIT_TIME, MAX_TS, R, dtype=np.float32)
    arelu = np.maximum(alpha.reshape(R).astype(np.float32), 0.0)

    Tt = np.ascontiguousarray(T.transpose(1, 0, 2)).astype(np.float32)
    Xb = np.ascontiguousarray(X.transpose(1, 0, 2)).astype(BF16)
    XPb = np.maximum(Xb, 0)
    Mb = np.ascontiguousarray(M.transpose(1, 0, 2)).astype(BF16)
    DTb = np.ascontiguousarray(DT.transpose(1, 0, 2)).astype(BF16)
    Pb = np.ascontiguousarray(P.transpose(1, 0, 2)).astype(BF16)
    id48 = np.eye(D, dtype=np.float32).astype(BF16)

    # WX[pair, l, k, rr, b16, d]: per r-pair, 7 weight channels expanded over
    # 16 b (b-independent; a 0-stride AP repeats them across all 32 b).
    wk_full = np.concatenate([w_t, 5.0 * b_t, w_v[..., None]], axis=3)  # [R, L, D, 7]
    in_maps = []
    for i in range(8):
        r0 = i * RL
        wx = wk_full[r0 : r0 + RL].transpose(1, 3, 0, 2)  # [L, 7, RL, D]
        wx = wx.reshape(L, 7, RL // 2, 2, D).transpose(2, 0, 1, 3, 4)  # [NP, L, 7, 2, D]
        wx = np.ascontiguousarray(
            np.broadcast_to(wx[:, :, :, :, None, :], (RL // 2, L, 7, 2, 1, D))
        ).astype(BF16)
        rn = np.broadcast_to(-refs[r0 : r0 + RL], (L, RL)).astype(np.float32)
        an = np.broadcast_to(-arelu[r0 : r0 + RL], (L, RL)).astype(np.float32)
        bvl = np.ascontiguousarray(
            (128.0 * b_v[r0 : r0 + RL, 0, :]).T
        ).astype(BF16)  # [D, RL]
        in_maps.append(
            {
                "Tt": Tt,
                "Xb": Xb,
                "XPb": XPb,
                "Mb": Mb,
                "DTb": DTb,
                "Pb": Pb,
                "WX": wx,
                "RN": np.ascontiguousarray(rn),
                "AN": np.ascontiguousarray(an),
                "BVl": bvl,
                "ID48": id48,
            }
        )
    return in_maps


def run(trace=False, **inputs):
    if "nc" not in _CACHE:
        _CACHE["nc"] = _build()
    nc = _CACHE["nc"]
    in_maps = _prep(**inputs)
    res = run_bass_kernel_spmd(nc, in_maps, core_ids=list(range(8)), trace=trace)
    out = np.empty((B, R, D), dtype=np.float32)
    for i in range(8):
        out[:, i * RL : (i + 1) * RL, :] = res.results[i]["out"]
    return out, res


def kernel(**inputs) -> np.ndarray:
    out, _ = run(trace=False, **inputs)
    return out


# revision 33
# speedup vs baseline: 1.1732x; 1.0011x over previous
"""ALNN layer on 8 TRN2 NeuronCores (Bass/Tile, SPMD — no collectives).

Math (per reference):
  ref_r = linspace(0, 48, 64);  a_r = relu(alpha_r)
  e[b,r,l,d]  = exp(-a_r * |T[b,l,d] - ref_r|)
  intensity   = relu(X * e) = relu(X) * e            (e > 0 always)
  p[b,r,l,d]  = w0*X + w1*relu(X)*e + w2*M + w3*DT + w4*P + 5*b_t[r,l,d]
  h           = relu(p)
  out[b,r,d]  = relu( sum_l w_v[r,l,d]*h + 128*b_v[r,d] )

Design (measured on HW, ~109us):
- Shard R=64 across the 8 cores (8 r each); inputs replicated; host
  concatenates the per-core [B, 8, D] outputs. No cross-core traffic.
- Layout: partition = L (=128), free = (r-pair, b, d); the main loop
  processes r in PAIRS so every DVE op runs at FD 3072 (bf16 2x_1P mode,
  ~1.75us/op — per-instruction overhead amortized).
- All 12 tensor-ops per pair run on VectorE in bf16. GPSIMD is deliberately
  unused: it shares SBUF ports with DVE and starves (25us+ op outliers).
- Weights are consumed via 5-D APs [l, r2, (b:32 stride-0), 1, d]: with the
  r-pair dim leading, the pure-broadcast operand keeps DVE's 2x mode (plain
  per-r [l, (b:0), d] broadcasts measured ~4x slower). Weight DMA is the raw
  0.7MB per core, split per (pair, channel) so each slice gates only its mul.
- ACT (ScalarE) computes dist=Abs(T-ref_r) and e=Exp(-a_r*dist) with per-r
  [P,1] bias/scale operands, plus the relu. f32 dist path for accuracy.
- The sum over L runs on the TensorEngine: per r, one-hot-column lhsT
  matmuls accumulate wv*h into a shared PSUM tile [8, b-chunk, D]; the
  128*b_v bias is folded in by an identity-rhs matmul that opens each
  accumulation group. Epilogue: relu(psum) on ACT, chunked DMA out.
"""
import sys

import numpy as np

if "/opt/trn_rl_repo" not in sys.path:
    sys.path.insert(0, "/opt/trn_rl_repo")

import ml_dtypes

from concourse import bacc, mybir
import concourse.tile as tile
from concourse.bass_utils import run_bass_kernel_spmd

BF16 = ml_dtypes.bfloat16
B, L, D = 32, 128, 48
R = 64
RL = R // 8  # r per core
INIT_TIME, MAX_TS = 0.0, 48.0

_CACHE = {}


def _build():
    nc = bacc.Bacc("TRN2", target_bir_lowering=False, debug=False, num_devices=8)
    f32, bf16 = mybir.dt.float32, mybir.dt.bfloat16
    AF = mybir.ActivationFunctionType

    # DRAM parameters (per-core shards / replicas)
    dTt = nc.dram_tensor("Tt", [L, B, D], f32, kind="ExternalInput").ap()
    dX = nc.dram_tensor("Xb", [L, B, D], bf16, kind="ExternalInput").ap()
    dXP = nc.dram_tensor("XPb", [L, B, D], bf16, kind="ExternalInput").ap()
    dM = nc.dram_tensor("Mb", [L, B, D], bf16, kind="ExternalInput").ap()
    dDT = nc.dram_tensor("DTb", [L, B, D], bf16, kind="ExternalInput").ap()
    dP = nc.dram_tensor("Pb", [L, B, D], bf16, kind="ExternalInput").ap()
    dWX = nc.dram_tensor("WX", [RL // 2, L, 7, 2, 1, D], bf16, kind="ExternalInput").ap()
    dRN = nc.dram_tensor("RN", [L, RL], f32, kind="ExternalInput").ap()
    dAN = nc.dram_tensor("AN", [L, RL], f32, kind="ExternalInput").ap()
    dBV = nc.dram_tensor("BVl", [D, RL], bf16, kind="ExternalInput").ap()
    dID = nc.dram_tensor("ID48", [D, D], bf16, kind="ExternalInput").ap()
    dOUT = nc.dram_tensor("out", [B, RL, D], f32, kind="ExternalOutput").ap()

    NCH = 4  # psum chunks of 8 b each (8*48 = 384 floats < 512/bank)
    BC = B // NCH

    with tile.TileContext(nc, pool_alloc_mode="queue") as tc:
        with (
            tc.tile_pool(name="const", bufs=1) as cpool,
            tc.tile_pool(name="work", bufs=2) as wpool,
            tc.tile_pool(name="psum", bufs=1, space="PSUM") as ppool,
            tc.tile_pool(name="outp", bufs=1) as opool,
        ):
            # ---- load persistent inputs (X first: XP gates the first DVE op)
            tX = cpool.tile([L, B, D], bf16, tag="X")
            nc.sync.dma_start(tX[:], dX)
            tXP = cpool.tile([L, B, D], bf16, tag="XP")
            nc.sync.dma_start(tXP[:], dXP)
            # pair-0 weights early: the first DVE ops gate on them
            wt0 = wpool.tile([L, 7, 2, 1, D], bf16, tag="wt", name="wt0", bufs=2)
            for k in (1, 0, 4):
                nc.sync.dma_start(wt0[:, k], dWX[0, :, k])
            tT = cpool.tile([L, B, D], f32, tag="T")
            nc.sync.dma_start(tT[:], dTt)
            tM = cpool.tile([L, B, D], bf16, tag="M")
            nc.sync.dma_start(tM[:], dM)
            tDT = cpool.tile([L, B, D], bf16, tag="DT")
            nc.sync.dma_start(tDT[:], dDT)
            tP = cpool.tile([L, B, D], bf16, tag="P")
            nc.sync.dma_start(tP[:], dP)
            tRN = cpool.tile([L, RL], f32, tag="RN")
            nc.sync.dma_start(tRN[:], dRN)
            tAN = cpool.tile([L, RL], f32, tag="AN")
            nc.sync.dma_start(tAN[:], dAN)
            tBV = cpool.tile([D, RL], bf16, tag="BV")
            nc.sync.dma_start(tBV[:], dBV)
            tID = cpool.tile([D, D], bf16, tag="ID")
            nc.sync.dma_start(tID[:], dID)


            # one-hot lhsT matrices: oh[l, j, m] = (m == j)
            tOH = cpool.tile([L, RL, RL], bf16, tag="OH")
            nc.vector.memset(tOH[:], 0.0)
            for j in range(RL):
                nc.vector.memset(tOH[:, j, j : j + 1], 1.0)

            # psum accumulators, one bank-sized chunk of (b, d) each
            psc = [
                ppool.tile([RL, BC, D], mybir.dt.float32, tag=f"ps{c}", name=f"ps{c}")
                for c in range(NCH)
            ]
            # open each accumulation group with the bias term:
            # ps[r, b, d] = 128*b_v[r, d] via lhsT=BVl[d', r], rhs=Id[d', (b d)]
            for c in range(NCH):
                nc.tensor.matmul(
                    psc[c][:],
                    tBV[:],
                    tID[:, None, :].to_broadcast((D, BC, D)),
                    start=True,
                    stop=False,
                )

            H = 1    # weights not expanded; pure 0-stride broadcast over b; 0-stride rep covers all 32
            NP = RL // 2  # r-pairs per core
            S5 = (L, 2, 32, H, D)
            g2 = lambda ap: ap.rearrange("l r (g h) d -> l r g h d", g=32)
            for jj in range(NP):
                if jj == 0:
                    wt = wt0
                    for k in (2, 3, 5, 6):
                        nc.sync.dma_start(wt[:, k], dWX[jj, :, k])
                else:
                    wt = wpool.tile([L, 7, 2, H, D], bf16, tag="wt", name=f"wt{jj}", bufs=2)
                    for k in (1, 0, 4, 2, 3, 5, 6):
                        nc.sync.dma_start(wt[:, k], dWX[jj, :, k])
                wop = lambda k: wt[:, k, :, None, :, :].to_broadcast(S5)

                dist = wpool.tile([L, 2, B, D], f32, tag="dist", bufs=2)
                ebf = wpool.tile([L, 2, B, D], bf16, tag="ebf", bufs=2)
                for rr in range(2):
                    j = 2 * jj + rr
                    nc.scalar.activation(
                        dist[:, rr], tT[:], AF.Abs, bias=tRN[:, j : j + 1]
                    )
                    nc.scalar.activation(
                        ebf[:, rr], dist[:, rr], AF.Exp, scale=tAN[:, j : j + 1]
                    )

                rep = lambda tin: g2(tin[:, None].to_broadcast((L, 2, B, D)))
                q = wpool.tile([L, 2, B, D], bf16, tag="q", bufs=2)
                nc.vector.tensor_mul(g2(q[:]), rep(tXP), wop(1))
                a0 = wpool.tile([L, 2, B, D], bf16, tag="aa", name=f"a0_{jj}", bufs=4)
                nc.vector.tensor_mul(g2(a0[:]), rep(tX), wop(0))
                a2 = wpool.tile([L, 2, B, D], bf16, tag="aa", name=f"a2_{jj}", bufs=4)
                nc.vector.tensor_mul(g2(a2[:]), rep(tM), wop(2))
                a3 = wpool.tile([L, 2, B, D], bf16, tag="aa", name=f"a3_{jj}", bufs=4)
                nc.vector.tensor_mul(g2(a3[:]), rep(tDT), wop(3))
                a4 = wpool.tile([L, 2, B, D], bf16, tag="aa", name=f"a4_{jj}", bufs=4)
                nc.vector.tensor_mul(g2(a4[:]), rep(tP), wop(4))

                t = wpool.tile([L, 2, B, D], bf16, tag="t", bufs=2)
                nc.vector.tensor_mul(t[:], q[:], ebf[:])
                s1 = wpool.tile([L, 2, B, D], bf16, tag="s", name=f"s1_{jj}", bufs=3)
                nc.vector.tensor_add(s1[:], a0[:], a4[:])
                s2 = wpool.tile([L, 2, B, D], bf16, tag="s", name=f"s2_{jj}", bufs=3)
                nc.vector.tensor_add(s2[:], a2[:], a3[:])
                s3 = wpool.tile([L, 2, B, D], bf16, tag="s", name=f"s3_{jj}", bufs=3)
                nc.vector.tensor_add(s3[:], s1[:], s2[:])
                s4 = wpool.tile([L, 2, B, D], bf16, tag="s", name=f"s4_{jj}", bufs=3)
                nc.vector.tensor_add(g2(s4[:]), g2(s3[:]), wop(5))
                p = wpool.tile([L, 2, B, D], bf16, tag="s", name=f"p_{jj}", bufs=3)
                nc.vector.tensor_add(p[:], s4[:], t[:])

                h = wpool.tile([L, 2, B, D], bf16, tag="h", bufs=2)
                nc.scalar.activation(h[:], p[:], AF.Relu)
                wh = wpool.tile([L, 2, B, D], bf16, tag="wh", bufs=2)
                nc.vector.tensor_mul(g2(wh[:]), g2(h[:]), wop(6))

                for rr in range(2):
                    j = 2 * jj + rr
                    for c in range(NCH):
                        nc.tensor.matmul(
                            psc[c][:],
                            tOH[:, j, :],
                            wh[:, rr, c * BC : (c + 1) * BC, :],
                            start=False,
                            stop=(j == RL - 1),
                        )

            # epilogue: relu(psum) -> sbuf f32, DMA out per chunk
            outf = opool.tile([RL, B, D], mybir.dt.float32, tag="outf")
            dOUTt = dOUT.transpose([1, 0, 2])
            for c in range(NCH):
                cs = slice(c * BC, (c + 1) * BC)
                nc.scalar.activation(outf[:, cs, :], psc[c][:], AF.Relu)
                nc.sync.dma_start(dOUTt[:, cs, :], outf[:, cs, :])

    nc.compile()
    return nc


def _prep(X, T, M, DT, P, alpha, w_t, b_t, w_v, b_v):
    """Host-side shard prep: returns in_maps for the 8 cores."""
    X, T, M, DT, P, alpha, w_t, b_t, w_v, b_v = (
        np.asarray(a) for a in (X, T, M, DT, P, alpha, w_t, b_t, w_v, b_v)
    )
    refs = np.linspace(INIT_TIME, MAX_TS, R, dtype=np.float32)
    arelu = np.maximum(alpha.reshape(R).astype(np.float32), 0.0)

    Tt = np.ascontiguousarray(T.transpose(1, 0, 2)).astype(np.float32)
    Xb = np.ascontiguousarray(X.transpose(1, 0, 2)).astype(BF16)
    XPb = np.maximum(Xb, 0)
    Mb = np.ascontiguousarray(M.transpose(1, 0, 2)).astype(BF16)
    DTb = np.ascontiguousarray(DT.transpose(1, 0, 2)).astype(BF16)
    Pb = np.ascontiguousarray(P.transpose(1, 0, 2)).astype(BF16)
    id48 = np.eye(D, dtype=np.float32).astype(BF16)

    # WX[pair, l, k, rr, b16, d]: per r-pair, 7 weight channels expanded over
    # 16 b (b-independent; a 0-stride AP repeats them across all 32 b).
    wk_full = np.concatenate([w_t, 5.0 * b_t, w_v[..., None]], axis=3)  # [R, L, D, 7]
    in_maps = []
    for i in range(8):
        r0 = i * RL
        wx = wk_full[r0 : r0 + RL].transpose(1, 3, 0, 2)  # [L, 7, RL, D]
        wx = wx.reshape(L, 7, RL // 2, 2, D).transpose(2, 0, 1, 3, 4)  # [NP, L, 7, 2, D]
        wx = np.ascontiguousarray(
            np.broadcast_to(wx[:, :, :, :, None, :], (RL // 2, L, 7, 2, 1, D))
        ).astype(BF16)
        rn = np.broadcast_to(-refs[r0 : r0 + RL], (L, RL)).astype(np.float32)
        an = np.broadcast_to(-arelu[r0 : r0 + RL], (L, RL)).astype(np.float32)
        bvl = np.ascontiguousarray(
            (128.0 * b_v[r0 : r0 + RL, 0, :]).T
        ).astype(BF16)  # [D, RL]
        in_maps.append(
            {
                "Tt": Tt,
                "Xb": Xb,
                "XPb": XPb,
                "Mb": Mb,
                "DTb": DTb,
                "Pb": Pb,
                "WX": wx,
                "RN": np.ascontiguousarray(rn),
                "AN": np.ascontiguousarray(an),
                "BVl": bvl,
                "ID48": id48,
            }
        )
    return in_maps


def run(trace=False, **inputs):
    if "nc" not in _CACHE:
        _CACHE["nc"] = _build()
    nc = _CACHE["nc"]
    in_maps = _prep(**inputs)
    res = run_bass_kernel_spmd(nc, in_maps, core_ids=list(range(8)), trace=trace)
    out = np.empty((B, R, D), dtype=np.float32)
    for i in range(8):
        out[:, i * RL : (i + 1) * RL, :] = res.results[i]["out"]
    return out, res


def kernel(**inputs) -> np.ndarray:
    out, _ = run(trace=False, **inputs)
    return out
